# revision 5
# baseline (speedup 1.0000x reference)
"""Segment mean-pool (LocalPooling1D) Trainium2 Bass kernel.

x [32, 8192, 256] f32, x_pos [32, 65] sorted int32 boundaries -> y [32, 64, 256].
y[b, j] = mean(x[b, x_pos[b,j]:x_pos[b,j+1]]), empty segments -> 0.

Strategy: data-parallel over batch, 4 rows per core on 8 cores. The kernel is
HBM-bound (x is 33.5 MB/core; 16 DMA queues x ~22.4 GB/s ~= 358 GB/s cap), so
the whole game is keeping the x DMA stream busy from t~=7us and letting the PE
track it with minimal start lag (any PE start lag survives to the end of the
run as a compute tail past the last DMA):

- pos row data goes out as a high-priority 1 KB descriptor on the sync queue.
- pos is broadcast across partitions with a ones[1,128].T @ pos[1,260] matmul
  on the (idle) PE instead of a slow gpsimd partition_broadcast; all indicator
  builds read the broadcast straight out of PSUM (the bank stays resident).
- S = (pos - p <= 128*ti) and ind = S[j] - S[j+1] are computed per HALF row,
  with BOTH halves' S written into the SAME tile: the write-after-read hazard
  forces the scheduler (which dispatches by ready time, not priority) to run
  S_a -> ind_a -> S_b -> ind_b, so the first matmuls start ~5us after pos.
- counts/reciprocal and their tiny DMAs are created after row 0's matmuls so
  they stay off the startup critical path.

Segment sums accumulate on the TensorEngine as psum += ind_tile.T @ x_tile in
fp32, with even/odd token-tiles packed into separate PE column groups.
y = psum * 1/max(cnt, 1).
"""

import os
import sys

import numpy as np

sys.path.insert(0, "/opt/trn_rl_repo")

import concourse.bacc as bacc
import concourse.bass as bass
import concourse.tile as tile
from concourse import mybir
from concourse.bass_utils import run_bass_kernel_spmd

dt = mybir.dt
Alu = mybir.AluOpType

# Problem constants (hardcoded per harness contract).
B, T, C, P = 32, 8192, 256, 65
NSEG = P - 1
NCORES = 8
R = B // NCORES          # batch rows per core
TOK = 128                # tokens per matmul tile (K)
KTILES = T // TOK        # 64 matmul tiles per row
HK = KTILES // 2         # token-tiles per indicator half

CFG = {
    "blk": int(os.environ.get("KB_BLK", "8")),            # token-tiles per x DMA
    "x_bufs": int(os.environ.get("KB_XBUFS", "14")),
    "ind_bufs": int(os.environ.get("KB_INDBUFS", "2")),
    "psum_bufs": int(os.environ.get("KB_PSUMBUFS", "2")),
}


def build_program(cfg=CFG):
    blk = cfg["blk"]
    nblk = KTILES // blk

    nc = bacc.Bacc("TRN2", target_bir_lowering=False, debug=False)

    x_d = nc.dram_tensor("x", [R, T, C], dt.float32, kind="ExternalInput")
    pos_d = nc.dram_tensor("x_pos", [R, P], dt.int32, kind="ExternalInput")
    y_d = nc.dram_tensor("y", [R, NSEG, C], dt.float32, kind="ExternalOutput")

    with tile.TileContext(nc) as tc:
        with (
            tc.tile_pool(name="const", bufs=1) as constp,
            tc.tile_pool(name="xp", bufs=cfg["x_bufs"]) as xp,
            tc.tile_pool(name="indp", bufs=cfg["ind_bufs"]) as indp,
            tc.tile_pool(name="outp", bufs=2) as outp,
            tc.tile_pool(name="psp", bufs=cfg["psum_bufs"], space="PSUM") as psp,
            tc.tile_pool(name="pos_psp", bufs=1, space="PSUM") as pos_psp,
        ):
            with tc.high_priority():
                # pos rows: first descriptor on the sync queue (ahead of x).
                pos_flat = constp.tile([1, R, P], dt.int32)
                nc.sync.dma_start(pos_flat[:], pos_d[None, :, :])

                # Tiny constants: partition index p, 128*ti per tile, ones row.
                p_iota = constp.tile([TOK, 1], dt.float32)
                nc.gpsimd.iota(p_iota[:], pattern=[[1, 1]], base=0,
                               channel_multiplier=1,
                               allow_small_or_imprecise_dtypes=True)
                ti_f = constp.tile([TOK, KTILES], dt.float32)
                nc.gpsimd.iota(ti_f[:], pattern=[[TOK, KTILES]], base=0,
                               channel_multiplier=0,
                               allow_small_or_imprecise_dtypes=True)
                ones_r = constp.tile([1, TOK], dt.float32)
                nc.vector.memset(ones_r[:], 1.0)

                # pos -> f32, partition-broadcast via PE: ones.T @ posf.
                posf_flat = constp.tile([1, R * P], dt.float32)
                nc.vector.tensor_copy(
                    posf_flat[:], pos_flat[:].rearrange("one r p -> one (r p)")
                )
                pos_ps = pos_psp.tile([TOK, R * P], dt.float32)
                nc.tensor.matmul(pos_ps[:], ones_r[:], posf_flat[:],
                                 start=True, stop=True)

            recip = constp.tile([NSEG, R], dt.float32)  # ops created in row 0

            for r in range(R):
                # S[p, ti, j] = (pos[j] - p <= 128*ti); ind = S[j] - S[j+1].
                # Both halves share one S tile: the second write must wait for
                # ind_a's read, which forces S_a -> ind_a -> S_b -> ind_b.
                S_t = indp.tile([TOK, HK, P], dt.float32, tag="s")
                halves = []
                for h in range(2):
                    nc.vector.scalar_tensor_tensor(
                        S_t[:],
                        pos_ps[:, r * P : (r + 1) * P][:, None, :].broadcast_to(
                            (TOK, HK, P)
                        ),
                        p_iota[:],
                        ti_f[:, h * HK : (h + 1) * HK, None].broadcast_to(
                            (TOK, HK, P)
                        ),
                        op0=Alu.subtract,
                        op1=Alu.is_le,
                    )
                    ind_h = indp.tile([TOK, HK, NSEG], dt.float32, tag=f"ind{h}")
                    nc.vector.tensor_tensor(
                        ind_h[:], S_t[:, :, 0:NSEG], S_t[:, :, 1:P],
                        op=Alu.subtract,
                    )
                    halves.append(ind_h)

                ps = psp.tile([2 * NSEG, C], dt.float32)
                xr = x_d[r].rearrange("(b k p) c -> b p k c", k=blk, p=TOK)
                for b in range(nblk):
                    xt = xp.tile([TOK, blk * C], dt.float32)
                    xt_v = xt[:].rearrange("p (k c) -> p k c", k=blk)
                    eng = nc.scalar if b % 2 else nc.sync
                    eng.dma_start(xt_v, xr[b])
                    for k in range(blk):
                        ti = b * blk + k
                        rhs = xt[:, k * C : (k + 1) * C]
                        lhsT = halves[ti // HK][:, ti % HK, :]
                        half = ti % 2
                        nc.tensor.matmul(
                            ps[half * NSEG : (half + 1) * NSEG, :], lhsT, rhs,
                            start=(ti == half), stop=(ti == KTILES - 2 + half),
                            tile_position=(0, half * NSEG),
                            skip_group_check=True,
                        )

                if r == 0:
                    # counts -> 1/max(cnt, 1) for all rows: [64, R]. Created
                    # here (not at the top) to stay off the startup path.
                    pos_lo = constp.tile([NSEG, R], dt.int32)
                    nc.gpsimd.dma_start(
                        pos_lo[:], pos_d[:, 0:NSEG].rearrange("r p -> p r")
                    )
                    pos_hi = constp.tile([NSEG, R], dt.int32)
                    nc.gpsimd.dma_start(
                        pos_hi[:], pos_d[:, 1:P].rearrange("r p -> p r")
                    )
                    cnt_f = constp.tile([NSEG, R], dt.float32)
                    nc.vector.tensor_tensor(cnt_f[:], pos_hi[:], pos_lo[:],
                                            op=Alu.subtract)
                    cntc = constp.tile([NSEG, R], dt.float32)
                    nc.vector.tensor_scalar(cntc[:], cnt_f[:], 1.0, None,
                                            op0=Alu.max)
                    nc.vector.reciprocal(recip[:], cntc[:])

                out_t = outp.tile([NSEG, C], dt.float32)
                # DVE reads one PSUM operand per op: scale each half alone.
                half_t = outp.tile([NSEG, C], dt.float32, tag="half")
                nc.vector.tensor_scalar(
                    half_t[:], ps[NSEG : 2 * NSEG, :], recip[:, r : r + 1], None,
                    op0=Alu.mult,
                )
                nc.vector.scalar_tensor_tensor(
                    out_t[:], ps[0:NSEG, :], recip[:, r : r + 1], half_t[:],
                    op0=Alu.mult, op1=Alu.add,
                )
                nc.gpsimd.dma_start(y_d[r], out_t[:])

    nc.compile()
    return nc


_PROGRAM = None


def _get_program():
    global _PROGRAM
    if _PROGRAM is None:
        _PROGRAM = build_program()
    return _PROGRAM


def kernel(x, x_pos):
    x = np.ascontiguousarray(x, dtype=np.float32)
    x_pos = np.ascontiguousarray(x_pos, dtype=np.int32)
    nc = _get_program()
    in_maps = [
        {"x": x[c * R : (c + 1) * R], "x_pos": x_pos[c * R : (c + 1) * R]}
        for c in range(NCORES)
    ]
    res = run_bass_kernel_spmd(nc, in_maps, list(range(NCORES)))
    y = np.concatenate([res.results[c]["y"] for c in range(NCORES)], axis=0)
    return y.astype(np.float32)
